# revision 9
# baseline (speedup 1.0000x reference)
"""Trainium2 Bass kernel for an edge-weighted two-layer sparse MLP (QBAF).

Math (identical to the gather/segment_sum reference):
    out = sigmoid(x @ W1 + b1) @ W2 + b2
with W1 [2048, 1024] / W2 [1024, 8] densified on host from the sparse
edge lists (scatter-add, duplicates accumulate like segment_sum).

Sharding: data-parallel over batch — 8 cores x 512 rows each; weights
replicated.

Per-core plan (PE-roofline oriented):
  - Layer 1 mixed precision: the first N8=12 contraction k-tiles run as
    6 fp8(e4m3) DoubleRow pair-tiles (2 k-tiles per matmul at the 2x
    fp8 rate), the remaining 4 k-tiles in fp16 at full rate. Host
    pre-scales x by 16 and W1 by 64 (keeps e4m3 away from subnormals;
    exact powers of two) so both phases accumulate at one PSUM scale;
    the sigmoid applies scale=1/1024 plus the b1 bias. Host-simulated
    rel err: 0.0160 (threshold 2e-2).
  - x-tiles stream on the scalar HWDGE ring, W-tiles on the sync ring
    (two rings so per-DMA descriptor-issue time is not the pacer), and
    the first pair's W is split so the m0/m1 stationary blocks land
    first: the PE starts real matmuls ~2.5us earlier than a fused
    layout allows, which also finishes the HAM clock ramp inside
    useful work.
  - fp8 phase is k-outer (tiles consumed in arrival order); the fp16
    phase is m-outer so acc[m] banks complete staggered and the
    full-width sigmoid chain on ACT overlaps the tail of layer 1.
  - Layer 2 in fp16 at full rate chases the sigmoids; one bias-add and
    one [8, 512] out DMA on the sync ring finish the kernel.
  - Post-build IR passes strip Tile's start barrier, register-init
    moves, dead const memsets, and all end-of-kernel drains except the
    sync drain that guards out-DMA completion.
"""

import sys

import numpy as np

if "/opt/trn_rl_repo" not in sys.path:
    sys.path.insert(0, "/opt/trn_rl_repo")

B = 4096
F = 2048
N1 = 1024
NT = 8
NCORES = 8
BSH = B // NCORES  # 512 batch rows per core
P = 128
K1 = F // P  # 16 contraction k-tiles
M1 = N1 // P  # 8 neuron tiles
K2 = N1 // P  # 8 contraction tiles, layer 2
LXW = BSH + N1  # fused row width: [xT | W1] = 1536

N8 = 12  # k-tiles in fp8 (even; rest fp16)
SX = 16.0  # x pre-scale
SW = 64.0  # W1 pre-scale
SINV = 1.0 / (SX * SW)
NWARM = 17  # PE clock-ramp warmup matmuls; must bridge continuously
            # to the first real matmul or the HAM ramp resets

_CACHE = {}


def _build(n8=N8, nwarm=NWARM):
    import concourse.bass as bass
    import concourse.mybir as mybir
    import concourse.tile as tile

    dt = mybir.dt
    DR = mybir.MatmulPerfMode.DoubleRow
    npair = n8 // 2
    n16 = K1 - n8

    nc = bass.Bass()
    x80 = nc.declare_dram_parameter("x80", [P, 2 * BSH], dt.float8e4, isOutput=False)
    w80 = nc.declare_dram_parameter("w80", [P, 2 * N1], dt.float8e4, isOutput=False)
    lx8 = nc.declare_dram_parameter("lx8", [(npair - 1) * P, 2 * LXW], dt.float8e4, isOutput=False)
    lxh = nc.declare_dram_parameter("lxh", [n16 * P, LXW], dt.float16, isOutput=False)
    w2p = nc.declare_dram_parameter("w2p", [P, K2 * NT], dt.float16, isOutput=False)
    cn = nc.declare_dram_parameter("cn", [P, M1 + 1], dt.float32, isOutput=False)
    outT = nc.declare_dram_parameter("outT", [NT, BSH], dt.float32, isOutput=True)

    with tile.TileContext(nc) as tc:
        with (
            tc.tile_pool(name="consts", bufs=1) as consts,
            tc.tile_pool(name="t0p", bufs=1) as t0p,
            tc.tile_pool(name="lx8p", bufs=max(npair - 1, 1)) as lx8p,
            tc.tile_pool(name="lxhp", bufs=max(n16, 1)) as lxhp,
            tc.tile_pool(name="hp", bufs=M1) as hp,
            tc.tile_pool(name="outp", bufs=1) as outp,
            tc.tile_pool(name="ps", bufs=8, space="PSUM") as ps,
        ):
            # --- DMAs: the first pair-tile is split (x, then W m-major in
            # two chunks) so the PE can start ~1.3us earlier; the remaining
            # tiles are fused [xT | W1] so every descriptor moves a fat 3KB
            # row. All big tiles ride the sync ring in consumption order;
            # consts ride the scalar ring.
            # Warmup scratch memzero is the FIRST scalar-engine instruction
            # so the PE clock-ramp warmup can start right after the engine
            # preambles (~5us), bridging continuously to the first real
            # matmul -- an idle PE resets the HAM ramp.
            wsc = consts.tile([P, BSH // 2], dt.float16, tag="wsc", name="wsc")
            nc.scalar.memzero(wsc[:])
            xt0 = t0p.tile([P, 2, BSH], dt.float8e4, tag="x0", name="x0")
            nc.sync.dma_start(out=xt0[:], in_=x80[:])
            wt0 = t0p.tile([P, M1, 2, P], dt.float8e4, tag="w0", name="w0")
            nc.sync.dma_start(out=wt0[:, 0 : M1 // 2], in_=w80[:, 0 : N1])
            nc.sync.dma_start(out=wt0[:, M1 // 2 :], in_=w80[:, N1:])
            w2s = consts.tile([P, K2 * NT], dt.float16, tag="w2", name="w2s")
            nc.scalar.dma_start(out=w2s[:], in_=w2p[:])
            cns = consts.tile([P, M1 + 1], dt.float32, tag="cn", name="cns")
            nc.scalar.dma_start(out=cns[:], in_=cn[:])
            # ACT pre-observes the cns DMA semaphore off the critical path
            # (hw allows one wait per ACT instruction, and the first sigmoid
            # already needs the PE wait).
            scr = consts.tile([P, 1], dt.float32, tag="scr", name="scr")
            nc.scalar.activation(
                scr[:], cns[:, 0:1], mybir.ActivationFunctionType.Copy
            )
            t8s = []
            for j in range(npair - 1):
                t = lx8p.tile([P, 2, LXW], dt.float8e4, tag="lx8", name=f"lx8_{j}")
                nc.sync.dma_start(out=t[:], in_=lx8[j * P : (j + 1) * P, :])
                t8s.append(t)
            ths = []
            for k in range(n16):
                t = lxhp.tile([P, LXW], dt.float16, tag="lxh", name=f"lxh_{k}")
                nc.sync.dma_start(out=t[:], in_=lxh[k * P : (k + 1) * P, :])
                ths.append(t)

            accs = [
                ps.tile([P, BSH], dt.float32, tag="acc", name=f"acc{m}")
                for m in range(M1)
            ]

            # --- HAM warm-up matmuls (see wsc memzero above).
            for _ in range(nwarm):
                nc.tensor.matmul(
                    accs[0][:, 0 : BSH // 2], wsc[:, 0:P], wsc[:],
                    start=True, stop=True, skip_group_check=True,
                )

            # --- Layer 1 fp8 phase, k-outer: 8 full-width DoubleRow matmuls
            # per pair-tile, one accumulation group per PSUM bank.
            for m in range(M1):
                nc.tensor.matmul(
                    accs[m][:],
                    wt0[:, m],
                    xt0[:],
                    start=True,
                    stop=False,
                    perf_mode=DR,
                    skip_group_check=True,
                )
            for j in range(npair - 1):
                for m in range(M1):
                    nc.tensor.matmul(
                        accs[m][:],
                        t8s[j][:, :, BSH + m * P : BSH + (m + 1) * P],
                        t8s[j][:, :, 0:BSH],
                        start=False,
                        stop=False,
                        perf_mode=DR,
                        skip_group_check=True,
                    )

            # --- fp16 phase, m-outer: acc[m] finishes after its 4 matmuls,
            # so sigmoids start ~6us before layer 1 ends and chase.
            hts = []
            for m in range(M1):
                for k in range(n16):
                    nc.tensor.matmul(
                        accs[m][:],
                        ths[k][:, BSH + m * P : BSH + (m + 1) * P],
                        ths[k][:, 0:BSH],
                        start=False,
                        stop=(k == n16 - 1),
                        skip_group_check=True,
                    )
                ht = hp.tile([P, BSH], dt.float16, tag="h", name=f"h{m}")
                nc.scalar.activation(
                    ht[:],
                    accs[m][:],
                    mybir.ActivationFunctionType.Sigmoid,
                    bias=cns[:, m : m + 1],
                    scale=SINV,
                )
                hts.append(ht)

            # --- Layer 2: full-width fp16, chases the sigmoid chain. acc2 is
            # the 9th psum tile -> reuses the bank freed by sigmoid 0.
            acc2 = ps.tile([P, BSH], dt.float32, tag="acc", name="acc2")
            for m in range(M1):
                nc.tensor.matmul(
                    acc2[:NT, :],
                    w2s[:, m * NT : (m + 1) * NT],
                    hts[m][:],
                    start=(m == 0),
                    stop=(m == M1 - 1),
                )
            outs = outp.tile([NT, BSH], dt.float32, tag="out", name="outs")
            nc.scalar.activation(
                outs[:],
                acc2[:NT, :],
                mybir.ActivationFunctionType.Identity,
                bias=cns[0:NT, M1 : M1 + 1],
                scale=1.0,
            )
            nc.sync.dma_start(out=outT[:], in_=outs[:])

    return nc


def _strip_start_barrier(nc):
    """Drop Tile's start-of-kernel all-engine drain + EVSEM barrier, the
    per-engine register-init moves (no hardware loops / predication in this
    kernel), and the never-read const-pool memsets from the 'main' block."""
    for fn in nc.m.functions:
        for bb in fn.blocks:
            if bb.name == "main":
                bb.instructions = [
                    i
                    for i in bb.instructions
                    if type(i).__name__
                    not in (
                        "InstDrain",
                        "InstEventSemaphore",
                        "InstRegisterMove",
                        "InstMemset",
                    )
                ]


def _slim_end_block(nc):
    """The Tile end block emits a Drain + barrier EventSemaphore pair per
    engine (paced by the slow gpsimd queue, ~2.4us of tail). Only the sync
    (SP) drain matters for correctness: it waits on the out-DMA completion
    semaphores so the NEFF cannot signal done with the transfer in flight.
    Each engine's own postamble runs in-order after its last real
    instruction, and every cross-engine semaphore a consumer waits on is
    produced earlier in its producer's queue, so the all-engine barrier is
    redundant."""
    from concourse import mybir

    for fn in nc.m.functions:
        for bb in fn.blocks:
            if bb.name.endswith("_end"):
                bb.instructions = [
                    i
                    for i in bb.instructions
                    if getattr(i, "engine", None) == mybir.EngineType.SP
                    and type(i).__name__ == "InstDrain"
                ]


def _legalize_single_wait(nc):
    """This neuronxcc build allows at most ONE sync wait per instruction.
    Split extras onto same-engine no-ops placed immediately before."""
    import bass_rust

    for fn in nc.m.functions:
        for bb in fn.blocks:
            out, changed = [], False
            for ins in bb.instructions:
                si = ins.sync_info
                waits = list(si.on_wait) if si is not None else []
                if len(waits) > 1:
                    for i, w in enumerate(waits[:-1]):
                        out.append(
                            bass_rust.InstNoOp(
                                name=f"{ins.name}-sw{i}",
                                engine=ins.engine,
                                ins=[],
                                outs=[],
                                sync_info=bass_rust.SyncInfo(
                                    on_wait=[w], on_update=[]
                                ),
                            )
                        )
                    ins.sync_info = bass_rust.SyncInfo(
                        on_wait=[waits[-1]], on_update=list(si.on_update)
                    )
                    changed = True
                out.append(ins)
            if changed:
                bb.instructions = out


def _densify(w, rows_in, cols_out, n_in, n_out):
    dense = np.zeros((n_in, n_out), np.float32)
    np.add.at(dense, (np.asarray(rows_in), np.asarray(cols_out)), np.asarray(w))
    return dense


def _prep_inputs(x, w1, b1, w2, b2, conn1_out, conn1_in, conn2_out, conn2_in, n8=N8):
    import ml_dtypes

    f8 = ml_dtypes.float8_e4m3fn
    npair = n8 // 2
    s8 = n8 * P
    x = np.asarray(x, dtype=np.float32)
    W1 = _densify(w1, conn1_in, conn1_out, F, N1)
    W2 = _densify(w2, conn2_in, conn2_out, N1, NT).astype(np.float16)
    # w2 packed k-major: w2p[p, m*NT + t] = W2[m*P + p, t]
    w2p = np.ascontiguousarray(
        W2.reshape(K2, P, NT).transpose(1, 0, 2).reshape(P, K2 * NT)
    )
    cn = np.zeros((P, M1 + 1), np.float32)
    cn[:, :M1] = np.asarray(b1, np.float32).reshape(M1, P).T
    cn[:NT, M1] = np.asarray(b2, np.float32)

    W1s = SW * W1  # [2048, 1024], scaled
    W8 = W1s[:s8].astype(f8)
    # first pair-tile W, m-major slot-pairs:
    #   w80[p, m*256 + i*128 + c] = W1s_fp8[128i + p, 128m + c]
    w80 = np.ascontiguousarray(
        W8[: 2 * P].reshape(2, P, M1, P).transpose(1, 2, 0, 3).reshape(P, 2 * N1)
    )
    xs = SX * x  # [4096, 2048], scaled
    in_maps = []
    for c in range(NCORES):
        xT = np.ascontiguousarray(xs[c * BSH : (c + 1) * BSH, :].T)  # [F, BSH]
        xT8 = xT[:s8].astype(f8)
        # first pair-tile x: x80[p, i*512 + b] = xT8[128i + p, b]
        x80 = np.ascontiguousarray(
            xT8[: 2 * P].reshape(2, P, BSH).transpose(1, 0, 2).reshape(P, 2 * BSH)
        )
        # fused pair-tiles j>=1: row (j*128+p) = [fused(256(j+1)+p) | fused(...+128)]
        V8 = np.concatenate([xT8[2 * P :], W8[2 * P :]], axis=1)  # [(n8-2)P, LXW]
        lx8 = np.ascontiguousarray(
            V8.reshape(npair - 1, 2, P, LXW).transpose(0, 2, 1, 3).reshape((npair - 1) * P, 2 * LXW)
        )
        Vh = np.concatenate([xT[s8:], W1s[s8:]], axis=1).astype(np.float16)
        lxh = np.ascontiguousarray(Vh)
        in_maps.append(
            {"x80": x80, "w80": w80, "lx8": lx8, "lxh": lxh, "w2p": w2p, "cn": cn}
        )
    return in_maps


def _run(inputs, l1_bf16=True, trace=False, n8=N8, nwarm=NWARM, **run_kwargs):
    """Build (cached), run on the 8 NeuronCores, gather. Returns
    (out [4096, 8] float32, BassKernelResults). l1_bf16 is accepted for
    test-harness compat and ignored (layer 1 is mixed fp8/fp16)."""
    from concourse.bass_utils import run_bass_kernel_spmd

    key = ("nc", n8, nwarm)
    if key not in _CACHE:
        nc = _build(n8, nwarm)
        _strip_start_barrier(nc)
        _slim_end_block(nc)
        _legalize_single_wait(nc)
        _CACHE[key] = nc
    nc = _CACHE[key]

    in_maps = _prep_inputs(**inputs, n8=n8)
    res = run_bass_kernel_spmd(
        nc, in_maps, list(range(NCORES)), trace=trace, **run_kwargs
    )
    out = np.empty((B, NT), np.float32)
    for c in range(NCORES):
        out[c * BSH : (c + 1) * BSH, :] = res.results[c]["outT"].T
    return out, res


def kernel(**inputs):
    out, _ = _run(inputs)
    return out
